# revision 5
# baseline (speedup 1.0000x reference)
"""TBCNN tree-convolution layer on 8 trn2 NeuronCores (data-parallel).

Math (validated against reference to 1.6e-7):
  res[b,n] = X[b,n]@w_t + P[b,n]@w_l + Q[b,n]@(w_r-w_l) + conv -> leaky_relu(0.01)
  P = S_P @ X, Q = S_Q @ X  with S_* (512x512) adjacency built from children:
  S_P[n,m] = sum_j has[n,j]*[c[n,j]=m];  S_Q[n,m] = sum_j w1[n,j]*[c[n,j]=m]
  w1 = has*(a*j + b*[j==0]); a = 1/(ns-1) if ns>1 else 0; b = 0.5*[ns==1]

Sharding: batch (tree) axis split 4 trees/core across 8 cores via pmap;
weights replicated. The gather is reformulated as dense adjacency matmuls
(each node referenced ~16x -> PE-friendly, no data-dependent addressing).
"""

import numpy as np

B, N, C, D, O = 32, 512, 16, 256, 256
NCORES = 8
TPC = B // NCORES

_compiled = None


def _host_prep(nodes, w_t, w_l, w_r, conv, children):
    nodes = np.asarray(nodes, np.float32)
    ch = np.asarray(children).astype(np.int64)
    has = ch > 0
    ns = has.sum(-1)
    a = np.where(ns > 1, 1.0 / np.maximum(ns - 1, 1), 0.0)
    bco = np.where(ns == 1, 0.5, 0.0)
    jar = np.arange(C, dtype=np.float64)
    w0 = has.astype(np.float64)
    w1 = has * (a[..., None] * jar + bco[..., None] * (jar == 0))

    bi, ni, ji = np.nonzero(has)
    mi = ch[bi, ni, ji]
    sp = np.zeros((B, N, N), np.float32)
    sq = np.zeros((B, N, N), np.float32)
    np.add.at(sp, (bi, ni, mi), w0[bi, ni, ji])
    np.add.at(sq, (bi, ni, mi), w1[bi, ni, ji])
    return nodes, sp, sq


def kernel(**inputs):
    global _compiled
    import jax
    import jax.numpy as jnp

    nodes, sp, sq = _host_prep(**inputs)
    w_t = np.asarray(inputs["w_t"], np.float32)
    w_l = np.asarray(inputs["w_l"], np.float32)
    w_rl = np.asarray(inputs["w_r"], np.float32) - w_l
    conv = np.asarray(inputs["conv"], np.float32)

    if _compiled is None:
        def per_core(x, s_p, s_q, wt, wl, wrl, cv):
            # x: (TPC,N,D)  s_*: (TPC,N,N)
            p = jnp.einsum("tnm,tmd->tnd", s_p, x)
            q = jnp.einsum("tnm,tmd->tnd", s_q, x)
            res = x @ wt + p @ wl + q @ wrl + cv
            return jnp.where(res > 0, res, 0.01 * res)

        _compiled = jax.pmap(
            per_core,
            in_axes=(0, 0, 0, None, None, None, None),
            devices=jax.devices()[:NCORES],
        )

    xs = nodes.reshape(NCORES, TPC, N, D)
    sps = sp.reshape(NCORES, TPC, N, N)
    sqs = sq.reshape(NCORES, TPC, N, N)
    out = _compiled(xs, sps, sqs, w_t, w_l, w_rl, conv)
    return np.asarray(out).reshape(B, N, O)
